# revision 12
# baseline (speedup 1.0000x reference)
"""Trainium2 Bass kernel for nn_GCNStacking: 3-layer dense-adjacency GraphConv.

Per batch element b (one per NeuronCore, B=8 = n_cores=8, pure data parallel):
    H = relu(A @ (X @ Wm0^T) + X @ Ws0^T + b0)
    H = relu(A @ (H @ Wm1^T) + H @ Ws1^T + b1)
    H =      A @ (H @ Wm2^T) + H @ Ws2^T + b2

v2 dataflow (per core), state transposed, Ht = H^T [C=64, N=2048]:
  - A row-slabs [128, 2048] stream from HBM via SWDGE cast-DMAs that land
    bf16 in SBUF (HBM still reads the full f32; SBUF write + all on-chip
    work is 2-byte).  GpSimd is dedicated to DMA emission.
  - Each slab is PE-transposed (16 bf16 tiles; FWL weight loads) into
    PSUM, then cast-copied (DVE/ACT 3:1) into resident A^T bf16
    [j%128, j-block, i].  A^T is reused by all 3 layers.
  - Aggregation: col-packed bf16 matmuls — even j-blocks accumulate into
    PSUM partitions 0:64, odd into 64:128, concurrently in the array's
    column groups; + self term; evacuated with bias+relu on ACT/DVE.
  - Layer-1 aggregation chunks run one chunk behind the slab transposes,
    inside the DMA window; mprod for the next layer trails each chunk.
  - Final layer transposed back to natural [N, C] via PE and DMA'd out
    chunk by chunk.

AGG_MODE env GCN_AGG_MODE: "bf16" (default; err ~3e-3) | "f32r" (exact-ish).
A_MODE env GCN_A_MODE: "swdge" (default; cast-DMA bf16 slabs) | "hwdge"
    (f32 slabs + f32 transposes, cast at the PSUM evac).
"""
import sys

for _p in ("/opt/trn_rl_repo",):
    if _p not in sys.path:
        sys.path.insert(0, _p)

import numpy as np
import orjson

import concourse.bass as bass
import concourse.tile as tile
from concourse import mybir
from concourse.bass import _add_dep_helper as add_dep

f32 = mybir.dt.float32
f32r = mybir.dt.float32r
bf16 = mybir.dt.bfloat16

import os as _os
AGG_MODE = _os.environ.get("GCN_AGG_MODE", "bf16")
A_MODE = _os.environ.get("GCN_A_MODE", "swdge")

# ---------------------------------------------------------------------------
# Workaround: this walrus build accepts at most ONE embedded sync-wait per
# instruction ("Too many sync wait commands").  Split excess waits onto
# inserted NoOps (same engine, right before the host instruction).
# ---------------------------------------------------------------------------
_ws_ctr = [0]


def _split_waits_json(bir_bytes: bytes) -> bytes:
    d = orjson.loads(bir_bytes)
    changed = False
    for fn in d.get("functions", []):
        for blk in fn.get("blocks", []):
            out = []
            for inst in blk.get("instructions", []):
                si = inst.get("sync_info")
                waits = (si or {}).get("on_wait") or []
                eng = inst.get("engine")
                if len(waits) > 1 and eng and eng != "Unassigned":
                    changed = True
                    for w in waits[:-1]:
                        _ws_ctr[0] += 1
                        out.append({
                            "name": f"I-wsplit-{_ws_ctr[0]}",
                            "opcode": "NoOp",
                            "engine": eng,
                            "ins": [],
                            "outs": [],
                            "sync_info": {"on_wait": [w], "on_update": []},
                        })
                    si["on_wait"] = waits[-1:]
                out.append(inst)
            blk["instructions"] = out
    return orjson.dumps(d) if changed else bir_bytes


def _install_waitsplit():
    from concourse import bass2jax, bass_utils
    if getattr(bass_utils, "_waitsplit_installed", False):
        return
    orig = bass_utils.compile_bir_kernel

    def patched(bir_json, tmpdir, neff_name="file.neff"):
        return orig(_split_waits_json(bytes(bir_json)), tmpdir, neff_name=neff_name)

    bass_utils.compile_bir_kernel = patched
    bass2jax.compile_bir_kernel = patched
    bass_utils._waitsplit_installed = True


_install_waitsplit()

# ---------------------------------------------------------------------------
# Kernel builder
# ---------------------------------------------------------------------------
P = 128
C = 64
N_LAYERS = 3


def build_gcn(nn_nodes: int = 2048):
    """Build the single-core Bass program; the same program runs SPMD on all
    8 cores with per-core (per-batch) inputs."""
    NN = nn_nodes
    NB = NN // P            # node blocks (16)
    CH = 512                # aggregation i-chunk (one PSUM bank of f32)
    IC = NN // CH           # i-chunks (4)
    SPC = CH // P           # slabs per chunk (4)

    use_bf16 = AGG_MODE == "bf16"
    use_swdge = use_bf16 and A_MODE == "swdge"
    rdt = bf16 if use_bf16 else f32r       # resident dtype for A^T/M/Ht
    adt = bf16 if use_swdge else f32       # A slab / transpose dtype

    nc = bass.Bass()
    X_in = nc.declare_dram_parameter("X", [NN, C], f32, isOutput=False)
    A_in = nc.declare_dram_parameter("A", [NN, NN], f32, isOutput=False)
    W_in = {}
    b_in = {}
    for l in range(N_LAYERS):
        W_in[(l, "m")] = nc.declare_dram_parameter(f"Wm{l}", [C, C], f32, isOutput=False)
        W_in[(l, "s")] = nc.declare_dram_parameter(f"Ws{l}", [C, C], f32, isOutput=False)
        b_in[l] = nc.declare_dram_parameter(f"b{l}", [C], f32, isOutput=False)
    H_out = nc.declare_dram_parameter("H", [NN, C], f32, isOutput=True)

    with tile.TileContext(nc) as tc:
        with (
            tc.tile_pool(name="const", bufs=1) as const,
            tc.tile_pool(name="ht_pool", bufs=2) as ht_pool,
            tc.tile_pool(name="mn_pool", bufs=2) as mn_pool,
            tc.tile_pool(name="slab_pool", bufs=6) as slab_pool,
            tc.tile_pool(name="u_pool", bufs=3) as u_pool,
            tc.tile_pool(name="hb_pool", bufs=4) as hb_pool,
            tc.tile_pool(name="ps_tr", bufs=3, space="PSUM") as ps_tr,
            tc.tile_pool(name="ps_o", bufs=2, space="PSUM") as ps_o,
            tc.tile_pool(name="ps_m", bufs=1, space="PSUM") as ps_m,
        ):
            # ---- phase 0: identity, then A slab DMAs (critical path) ------
            ident = const.tile([P, P], f32, name="ident")
            id_i1 = nc.gpsimd.memset(ident, 0.0)
            id_i2 = nc.gpsimd.affine_select(
                out=ident, in_=ident,
                compare_op=mybir.AluOpType.not_equal,
                fill=1.0, base=0, pattern=[[-1, P]], channel_multiplier=1,
            )

            # A slabs: full rows [128, 2048]; SWDGE casts f32->bf16 in the
            # DMA so SBUF holds 2-byte slabs.  GpSimd does nothing else, so
            # its queue never blocks descriptor prefetch.
            slabs = []
            for s in range(NB):
                a_sl = slab_pool.tile([P, NN], adt, name="a_sl", tag="aslab")
                if use_swdge:
                    dma = nc.gpsimd.dma_start(a_sl, A_in[s * P:(s + 1) * P, :])
                else:
                    dma = nc.sync.dma_start(a_sl, A_in[s * P:(s + 1) * P, :])
                slabs.append((a_sl, dma))

            x_sb = const.tile([P, NB, C], f32, name="x_sb")
            x_dma = nc.sync.dma_start(
                x_sb, X_in[:].rearrange("(nb p) c -> p nb c", p=P))

            w_stage = {}
            w_dmas = []
            for l in range(N_LAYERS):
                for kind in ("m", "s"):
                    wst = const.tile([C, C], f32, name=f"wst_{l}{kind}")
                    w_dmas.append(nc.sync.dma_start(wst, W_in[(l, kind)][:]))
                    w_stage[(l, kind)] = wst
            b_sb = {}
            for l in range(N_LAYERS):
                bt = const.tile([C, 1], f32, name=f"b_sb{l}")
                nc.sync.dma_start(bt, b_in[l][:].rearrange("(p o) -> p o", o=1))
                b_sb[l] = bt

            # bf16 identity for the A transposes (cast once on DVE)
            if adt == bf16:
                ident_a = const.tile([P, P], bf16, name="ident_a")
                ida = nc.vector.tensor_copy(ident_a, ident)
            else:
                ident_a = ident

            # gate: one PE nop absorbing phase-0 input waits so the f32
            # transposes below carry at most one embedded wait each
            gate0 = nc.tensor.nop(nofuse=True)
            for d in (id_i1, id_i2, x_dma, *w_dmas):
                add_dep(gate0.ins, d.ins, True, "phase0 gate")

            # warm-up matmuls on the identity: engage the PE HAM clock-gate
            # (~3.4us of sustained activity -> 2.4 GHz) before real data
            # arrives, so the first A transposes run at full clock
            warm_gate = nc.tensor.nop(nofuse=True)
            for d in (id_i1, id_i2):
                add_dep(warm_gate.ins, d.ins, True, "warmup gate")
            pwarm = ps_m.tile([P, C], f32, name="pwarm", tag="m", bufs=2)
            for wi in range(28):
                wmm = nc.tensor.matmul(pwarm[:C, :], ident[:, :C],
                                       ident[:, :C], start=True, stop=True,
                                       skip_group_check=True)
                if wi == 0:
                    add_dep(wmm.ins, warm_gate.ins, False, "after warmup gate")

            # Ht[l]: transposed state [C, NN]; Ht[0] = X^T.
            # All transposes below are REGULAR matmuls against the identity
            # (lhsT^T @ I = lhsT^T): same result/cost-shape as PE
            # transpose-mode, but they count as PE-busy for the HAM clock
            # gate (transpose-mode doesn't -> K=4/8 half-clock) and bf16
            # weight loads get FWL.
            Ht = [ht_pool.tile([C, NN], rdt, name=f"Ht{l}", tag="ht")
                  for l in range(N_LAYERS)]
            for nb in range(NB):
                pt = ps_tr.tile([P, P], f32, name="pt_x", tag="trx", bufs=2)
                t = nc.tensor.matmul(pt[:C, :P], x_sb[:, nb, :], ident,
                                     start=True, stop=True)
                add_dep(t.ins, gate0.ins, False, "after gate0")
                nc.vector.tensor_copy(Ht[0][:, nb * P:(nb + 1) * P], pt[:C, :P])

            wT = {}
            for (l, kind), wst in w_stage.items():
                pw = ps_tr.tile([P, P], f32, name="pt_w", tag="trx", bufs=2)
                t = nc.tensor.matmul(pw[:C, :C], wst, ident[:C, :C],
                                     start=True, stop=True)
                add_dep(t.ins, gate0.ins, False, "after gate0")
                wt = const.tile([C, C], rdt, name=f"wT_{l}{kind}")
                nc.vector.tensor_copy(wt, pw[:C, :C])
                wT[(l, kind)] = wt

            # resident A^T [j-partition, j-block, i]
            ATr = const.tile([P, NB, NN], rdt, name="ATr")

            def emit_mprod(l, mn, jbs=None):
                """M_l natural [N, C] blocks: lhsT = Ht[l] block, rhs = WmT."""
                for jb in (range(NB) if jbs is None else jbs):
                    pm = ps_m.tile([P, C], f32, name="pm", tag="m", bufs=2)
                    nc.tensor.matmul(pm, Ht[l][:, jb * P:(jb + 1) * P],
                                     wT[(l, "m")], start=True, stop=True)
                    # Mn copies on Scalar (ACT) to keep DVE free
                    nc.scalar.copy(mn[:, jb, :], pm)

            def emit_evac(l, g, po):
                if use_bf16:
                    # col-packed halves: out = po[0:64] + po[64:128] + b.
                    # Engines read at most one non-scalar PSUM input per op.
                    v = u_pool.tile([C, CH], f32, name="v", tag="v")
                    nc.scalar.activation(v, po[C:2 * C, :],
                                         mybir.ActivationFunctionType.Identity,
                                         bias=b_sb[l], scale=1.0)
                    if l < N_LAYERS - 1:
                        u = u_pool.tile([C, CH], f32, name="u", tag="u")
                        nc.vector.tensor_tensor(u, po[:C, :], v,
                                                mybir.AluOpType.add)
                        nc.vector.tensor_scalar(
                            Ht[l + 1][:, g * CH:(g + 1) * CH], u,
                            0.0, None, mybir.AluOpType.max)
                        return
                    ho = u_pool.tile([C, CH], f32, name="ho", tag="ho")
                    nc.vector.tensor_tensor(ho, po[:C, :], v,
                                            mybir.AluOpType.add)
                else:
                    if l < N_LAYERS - 1:
                        nc.scalar.activation(
                            Ht[l + 1][:, g * CH:(g + 1) * CH], po[:C, :],
                            mybir.ActivationFunctionType.Relu,
                            bias=b_sb[l], scale=1.0)
                        return
                    ho = u_pool.tile([C, CH], f32, name="ho", tag="ho")
                    nc.scalar.activation(ho, po[:C, :],
                                         mybir.ActivationFunctionType.Identity,
                                         bias=b_sb[l], scale=1.0)
                # final layer: back to natural layout and out to DRAM
                for k in range(CH // P):
                    ph = ps_tr.tile([P, P], f32, name="ph", tag="trx", bufs=2)
                    nc.tensor.matmul(ph[:, :C], ho[:, k * P:(k + 1) * P],
                                     ident[:C, :C], start=True, stop=True)
                    hb = hb_pool.tile([P, C], f32, name="hb", tag="hb")
                    nc.vector.tensor_copy(hb, ph[:, :C])
                    r0 = g * CH + k * P
                    nc.sync.dma_start(H_out[r0:r0 + P, :], hb)

            def cast_copy(eng_idx, dst, srcp):
                # 3:1 DVE:ACT — ACT copies are ~1.6x slower and ACT also
                # carries the Mn copies and evacuations
                if eng_idx % 4 != 3:
                    nc.vector.tensor_copy(dst, srcp)
                else:
                    nc.scalar.copy(dst, srcp)

            def emit_slab_transposes(s):
                """Transpose one A row-slab (16 tiles, pairs into PSUM) into
                ATr columns i in [s*128, (s+1)*128)."""
                a_sl, dma = slabs[s]
                gate = nc.tensor.nop(nofuse=True)
                add_dep(gate.ins, dma.ins, True, "slab gate")
                for jp in range(NB // 2):
                    pt = ps_tr.tile([P, 2 * P], f32, name="pt_a", tag="tr",
                                    bufs=2)
                    for h in range(2):
                        jb = 2 * jp + h
                        t = nc.tensor.matmul(
                            pt[:, h * P:(h + 1) * P],
                            a_sl[:, jb * P:(jb + 1) * P],
                            ident_a, start=True, stop=True,
                            skip_group_check=True)
                        add_dep(t.ins, gate.ins, False, "after slab gate")
                    # pair copy: ATr[:, 2jp:2jp+2, s*128:(s+1)*128]
                    cast_copy(jp + s, ATr[:, 2 * jp:2 * jp + 2,
                                          s * P:(s + 1) * P], pt)

            # agg chunk split into emission quarters for interleaving
            open_po = {}

            def emit_agg_part(l, g, mn, part):
                """part 0..3: col-packed pairs; part 3 adds self term+evac."""
                cs = slice(g * CH, (g + 1) * CH)
                if use_bf16:
                    # col-packed: even j-blocks -> partitions 0:64, odd ->
                    # 64:128; concurrent in the array's column groups
                    if part == 0:
                        po = ps_o.tile([P, CH], f32, name="po", tag="o")
                        open_po[(l, g)] = po
                    else:
                        po = open_po[(l, g)]
                    for jb in range(4 * part, 4 * part + 4):
                        h = jb % 2
                        nc.tensor.matmul(
                            po[h * C:(h + 1) * C, :], mn[:, jb, :],
                            ATr[:, jb, cs],
                            start=(jb < 2), stop=(h == 1 and jb == NB - 1),
                            skip_group_check=True)
                    if part == 3:
                        del open_po[(l, g)]
                        nc.tensor.matmul(
                            po[:C, :], wT[(l, "s")], Ht[l][:, cs],
                            start=False, stop=True, skip_group_check=True)
                        emit_evac(l, g, po)
                else:
                    if part == 0:
                        po = ps_o.tile([C, CH], f32, name="po", tag="o")
                        open_po[(l, g)] = po
                    else:
                        po = open_po[(l, g)]
                    for jb in range(4 * part, 4 * part + 4):
                        nc.tensor.matmul(
                            po, mn[:, jb, :], ATr[:, jb, cs],
                            start=(jb == 0), stop=False,
                            skip_group_check=True)
                    if part == 3:
                        del open_po[(l, g)]
                        nc.tensor.matmul(
                            po, wT[(l, "s")], Ht[l][:, cs],
                            start=False, stop=True, skip_group_check=True)
                        emit_evac(l, g, po)

            # ---- layer 1, pipelined with the A load/transpose -------------
            mns = {}
            for l in range(N_LAYERS):
                mns[l] = mn_pool.tile([P, NB, C], rdt, name="mn", tag="mn")

            # mprod L0 spread over the first slabs to keep HAM warm between
            # transpose batches (transpose-mode alone doesn't count as
            # PE-busy for the clock gate)
            for s in range(NB):
                emit_slab_transposes(s)
                if s < 4:
                    emit_mprod(0, mns[0], range(4 * s, 4 * s + 4))
                else:
                    # one chunk behind the transposes
                    emit_agg_part(0, (s - 4) // SPC, mns[0], (s - 4) % SPC)
            # tail chunk of layer 1 + chunk-wise mprod for layer 2
            for part in range(4):
                emit_agg_part(0, IC - 1, mns[0], part)
            emit_mprod(1, mns[1])

            # ---- layers 2..3 ---------------------------------------------
            for l in range(1, N_LAYERS):
                for g in range(IC):
                    for part in range(4):
                        emit_agg_part(l, g, mns[l], part)
                    if l + 1 < N_LAYERS:
                        jb0 = g * (NB // IC)
                        emit_mprod(l + 1, mns[l + 1],
                                   range(jb0, jb0 + NB // IC))

    return nc


# ---------------------------------------------------------------------------
# Harness entry point
# ---------------------------------------------------------------------------
_NC_CACHE = {}


def _get_nc(nn_nodes):
    if nn_nodes not in _NC_CACHE:
        _NC_CACHE[nn_nodes] = build_gcn(nn_nodes)
    return _NC_CACHE[nn_nodes]


def kernel(X, A, Wm0, Ws0, b0, Wm1, Ws1, b1, Wm2, Ws2, b2, _trace=False):
    from concourse.bass_utils import run_bass_kernel_spmd

    X = np.ascontiguousarray(np.asarray(X, dtype=np.float32))
    A = np.ascontiguousarray(np.asarray(A, dtype=np.float32))
    B, NN, _C = X.shape
    assert B == 8, f"expected batch 8 (one per core), got {B}"

    shared = {
        "Wm0": np.ascontiguousarray(np.asarray(Wm0, np.float32)),
        "Ws0": np.ascontiguousarray(np.asarray(Ws0, np.float32)),
        "b0": np.ascontiguousarray(np.asarray(b0, np.float32)),
        "Wm1": np.ascontiguousarray(np.asarray(Wm1, np.float32)),
        "Ws1": np.ascontiguousarray(np.asarray(Ws1, np.float32)),
        "b1": np.ascontiguousarray(np.asarray(b1, np.float32)),
        "Wm2": np.ascontiguousarray(np.asarray(Wm2, np.float32)),
        "Ws2": np.ascontiguousarray(np.asarray(Ws2, np.float32)),
        "b2": np.ascontiguousarray(np.asarray(b2, np.float32)),
    }
    nc = _get_nc(NN)
    in_maps = [dict(shared, X=X[b], A=A[b]) for b in range(B)]
    res = run_bass_kernel_spmd(nc, in_maps, core_ids=list(range(B)),
                               trace=_trace)
    out = np.stack([res.results[b]["H"] for b in range(B)], axis=0)
    if _trace:
        return out, res
    return out


# revision 18
# speedup vs baseline: 1.1300x; 1.1300x over previous
"""Trainium2 Bass kernel for nn_GCNStacking: 3-layer dense-adjacency GraphConv.

Per batch element b (one per NeuronCore, B=8 = n_cores=8, pure data parallel):
    H = relu(A @ (X @ Wm0^T) + X @ Ws0^T + b0)
    H = relu(A @ (H @ Wm1^T) + H @ Ws1^T + b1)
    H =      A @ (H @ Wm2^T) + H @ Ws2^T + b2

v2 dataflow (per core), state transposed, Ht = H^T [C=64, N=2048]:
  - A row-slabs [128, 2048] stream from HBM via SWDGE cast-DMAs that land
    bf16 in SBUF (HBM still reads the full f32; SBUF write + all on-chip
    work is 2-byte).  GpSimd is dedicated to DMA emission.
  - Each slab is PE-transposed (16 bf16 tiles; FWL weight loads) into
    PSUM, then cast-copied (DVE/ACT 3:1) into resident A^T bf16
    [j%128, j-block, i].  A^T is reused by all 3 layers.
  - Aggregation: col-packed bf16 matmuls — even j-blocks accumulate into
    PSUM partitions 0:64, odd into 64:128, concurrently in the array's
    column groups; + self term; evacuated with bias+relu on ACT/DVE.
  - Layer-1 aggregation chunks run one chunk behind the slab transposes,
    inside the DMA window; mprod for the next layer trails each chunk.
  - Final layer transposed back to natural [N, C] via PE and DMA'd out
    chunk by chunk.

AGG_MODE env GCN_AGG_MODE: "bf16" (default; err ~3e-3) | "f32r" (exact-ish).
A_MODE env GCN_A_MODE: "swdge" (default; cast-DMA bf16 slabs) | "hwdge"
    (f32 slabs + f32 transposes, cast at the PSUM evac).
"""
import sys

for _p in ("/opt/trn_rl_repo",):
    if _p not in sys.path:
        sys.path.insert(0, _p)

import numpy as np
import orjson

import concourse.bass as bass
import concourse.tile as tile
from concourse import mybir
from concourse.bass import _add_dep_helper as add_dep

f32 = mybir.dt.float32
f32r = mybir.dt.float32r
bf16 = mybir.dt.bfloat16

import os as _os
AGG_MODE = _os.environ.get("GCN_AGG_MODE", "bf16")
A_MODE = _os.environ.get("GCN_A_MODE", "swdge")

# ---------------------------------------------------------------------------
# Workaround: this walrus build accepts at most ONE embedded sync-wait per
# instruction ("Too many sync wait commands").  Split excess waits onto
# inserted NoOps (same engine, right before the host instruction).
# ---------------------------------------------------------------------------
_ws_ctr = [0]


def _split_waits_json(bir_bytes: bytes) -> bytes:
    d = orjson.loads(bir_bytes)
    changed = False
    for fn in d.get("functions", []):
        for blk in fn.get("blocks", []):
            out = []
            for inst in blk.get("instructions", []):
                si = inst.get("sync_info")
                waits = (si or {}).get("on_wait") or []
                eng = inst.get("engine")
                if len(waits) > 1 and eng and eng != "Unassigned":
                    changed = True
                    for w in waits[:-1]:
                        _ws_ctr[0] += 1
                        out.append({
                            "name": f"I-wsplit-{_ws_ctr[0]}",
                            "opcode": "NoOp",
                            "engine": eng,
                            "ins": [],
                            "outs": [],
                            "sync_info": {"on_wait": [w], "on_update": []},
                        })
                    si["on_wait"] = waits[-1:]
                out.append(inst)
            blk["instructions"] = out
    return orjson.dumps(d) if changed else bir_bytes


def _install_waitsplit():
    from concourse import bass2jax, bass_utils
    if getattr(bass_utils, "_waitsplit_installed", False):
        return
    orig = bass_utils.compile_bir_kernel

    def patched(bir_json, tmpdir, neff_name="file.neff"):
        return orig(_split_waits_json(bytes(bir_json)), tmpdir, neff_name=neff_name)

    bass_utils.compile_bir_kernel = patched
    bass2jax.compile_bir_kernel = patched
    bass_utils._waitsplit_installed = True


_install_waitsplit()

# ---------------------------------------------------------------------------
# Kernel builder
# ---------------------------------------------------------------------------
P = 128
C = 64
N_LAYERS = 3


def build_gcn(nn_nodes: int = 2048):
    """Build the single-core Bass program; the same program runs SPMD on all
    8 cores with per-core (per-batch) inputs."""
    NN = nn_nodes
    NB = NN // P            # node blocks (16)
    CH = 512                # aggregation i-chunk (one PSUM bank of f32)
    IC = NN // CH           # i-chunks (4)
    SPC = CH // P           # slabs per chunk (4)

    use_bf16 = AGG_MODE == "bf16"
    use_swdge = use_bf16 and A_MODE == "swdge"
    rdt = bf16 if use_bf16 else f32r       # resident dtype for A^T/M/Ht
    adt = bf16 if use_swdge else f32       # A slab / transpose dtype
    HS = NN // 2                           # half-slab width (1024)

    nc = bass.Bass()
    X_in = nc.declare_dram_parameter("X", [NN, C], f32, isOutput=False)
    A_in = nc.declare_dram_parameter("A", [NN, NN], f32, isOutput=False)
    W_in = {}
    b_in = {}
    for l in range(N_LAYERS):
        W_in[(l, "m")] = nc.declare_dram_parameter(f"Wm{l}", [C, C], f32, isOutput=False)
        W_in[(l, "s")] = nc.declare_dram_parameter(f"Ws{l}", [C, C], f32, isOutput=False)
        b_in[l] = nc.declare_dram_parameter(f"b{l}", [C], f32, isOutput=False)
    H_out = nc.declare_dram_parameter("H", [NN, C], f32, isOutput=True)

    with tile.TileContext(nc) as tc:
        with (
            tc.tile_pool(name="const", bufs=1) as const,
            tc.tile_pool(name="ht_pool", bufs=2) as ht_pool,
            tc.tile_pool(name="mn_pool", bufs=2) as mn_pool,
            tc.tile_pool(name="slab_pool", bufs=12) as slab_pool,
            tc.tile_pool(name="u_pool", bufs=3) as u_pool,
            tc.tile_pool(name="hb_pool", bufs=4) as hb_pool,
            tc.tile_pool(name="ps_tr", bufs=3, space="PSUM") as ps_tr,
            tc.tile_pool(name="ps_o", bufs=2, space="PSUM") as ps_o,
            tc.tile_pool(name="ps_m", bufs=1, space="PSUM") as ps_m,
        ):
            # ---- phase 0: identity, then A slab DMAs (critical path) ------
            ident = const.tile([P, P], f32, name="ident")
            id_i1 = nc.gpsimd.memset(ident, 0.0)
            id_i2 = nc.gpsimd.affine_select(
                out=ident, in_=ident,
                compare_op=mybir.AluOpType.not_equal,
                fill=1.0, base=0, pattern=[[-1, P]], channel_multiplier=1,
            )

            # A row half-slabs [128, 1024] (512 KB f32): HWDGE 1MB-class DMAs
            # run near line rate; SWDGE cast-DMAs measured only ~190 GB/s.
            halves = {}
            for s in range(NB):
                for h in range(2):
                    a_sl = slab_pool.tile([P, HS], adt, name="a_sl",
                                          tag="aslab")
                    src = A_in[s * P:(s + 1) * P, h * HS:(h + 1) * HS]
                    if use_swdge:
                        dma = nc.gpsimd.dma_start(a_sl, src)
                    else:
                        dma = nc.sync.dma_start(a_sl, src)
                    halves[(s, h)] = (a_sl, dma)

            x_sb = const.tile([P, NB, C], f32, name="x_sb")
            x_dma = nc.sync.dma_start(
                x_sb, X_in[:].rearrange("(nb p) c -> p nb c", p=P))

            w_stage = {}
            w_dmas = []
            for l in range(N_LAYERS):
                for kind in ("m", "s"):
                    wst = const.tile([C, C], f32, name=f"wst_{l}{kind}")
                    w_dmas.append(nc.sync.dma_start(wst, W_in[(l, kind)][:]))
                    w_stage[(l, kind)] = wst
            b_sb = {}
            for l in range(N_LAYERS):
                bt = const.tile([C, 1], f32, name=f"b_sb{l}")
                nc.sync.dma_start(bt, b_in[l][:].rearrange("(p o) -> p o", o=1))
                b_sb[l] = bt

            # bf16 identity for the A transposes (cast once on DVE)
            if adt == bf16:
                ident_a = const.tile([P, P], bf16, name="ident_a")
                ida = nc.vector.tensor_copy(ident_a, ident)
            else:
                ident_a = ident

            # gate: one PE nop absorbing phase-0 input waits so the f32
            # transposes below carry at most one embedded wait each
            gate0 = nc.tensor.nop(nofuse=True)
            for d in (id_i1, id_i2, x_dma, *w_dmas):
                add_dep(gate0.ins, d.ins, True, "phase0 gate")

            # warm-up matmuls on the identity: engage the PE HAM clock-gate
            # (~3.4us of sustained activity -> 2.4 GHz) before real data
            # arrives, so the first A transposes run at full clock
            warm_gate = nc.tensor.nop(nofuse=True)
            for d in (id_i1, id_i2):
                add_dep(warm_gate.ins, d.ins, True, "warmup gate")
            pwarm = ps_m.tile([P, C], f32, name="pwarm", tag="m", bufs=2)
            for wi in range(28):
                wmm = nc.tensor.matmul(pwarm[:C, :], ident[:, :C],
                                       ident[:, :C], start=True, stop=True,
                                       skip_group_check=True)
                if wi == 0:
                    add_dep(wmm.ins, warm_gate.ins, False, "after warmup gate")

            # Ht[l]: transposed state [C, NN]; Ht[0] = X^T.
            # All transposes below are REGULAR matmuls against the identity
            # (lhsT^T @ I = lhsT^T): same result/cost-shape as PE
            # transpose-mode, but they count as PE-busy for the HAM clock
            # gate (transpose-mode doesn't -> K=4/8 half-clock) and bf16
            # weight loads get FWL.
            Ht = [ht_pool.tile([C, NN], rdt, name=f"Ht{l}", tag="ht")
                  for l in range(N_LAYERS)]
            for nb in range(NB):
                pt = ps_tr.tile([P, P], f32, name="pt_x", tag="trx", bufs=2)
                t = nc.tensor.matmul(pt[:C, :P], x_sb[:, nb, :], ident,
                                     start=True, stop=True)
                add_dep(t.ins, gate0.ins, False, "after gate0")
                nc.vector.tensor_copy(Ht[0][:, nb * P:(nb + 1) * P], pt[:C, :P])

            wT = {}
            for (l, kind), wst in w_stage.items():
                pw = ps_tr.tile([P, P], f32, name="pt_w", tag="trx", bufs=2)
                t = nc.tensor.matmul(pw[:C, :C], wst, ident[:C, :C],
                                     start=True, stop=True)
                add_dep(t.ins, gate0.ins, False, "after gate0")
                wt = const.tile([C, C], rdt, name=f"wT_{l}{kind}")
                nc.vector.tensor_copy(wt, pw[:C, :C])
                wT[(l, kind)] = wt

            # resident A^T [j-partition, j-block, i]
            ATr = const.tile([P, NB, NN], rdt, name="ATr")

            def emit_mprod(l, mn, jbs=None):
                """M_l natural [N, C] blocks: lhsT = Ht[l] block, rhs = WmT."""
                for jb in (range(NB) if jbs is None else jbs):
                    pm = ps_m.tile([P, C], f32, name="pm", tag="m", bufs=2)
                    nc.tensor.matmul(pm, Ht[l][:, jb * P:(jb + 1) * P],
                                     wT[(l, "m")], start=True, stop=True)
                    # Mn copies on Scalar (ACT) to keep DVE free
                    nc.scalar.copy(mn[:, jb, :], pm)

            def emit_evac(l, g, po):
                if use_bf16:
                    # col-packed halves: out = po[0:64] + po[64:128] + b.
                    # Engines read at most one non-scalar PSUM input per op.
                    v = u_pool.tile([C, CH], f32, name="v", tag="v")
                    nc.scalar.activation(v, po[C:2 * C, :],
                                         mybir.ActivationFunctionType.Identity,
                                         bias=b_sb[l], scale=1.0)
                    if l < N_LAYERS - 1:
                        u = u_pool.tile([C, CH], f32, name="u", tag="u")
                        nc.vector.tensor_tensor(u, po[:C, :], v,
                                                mybir.AluOpType.add)
                        nc.vector.tensor_scalar(
                            Ht[l + 1][:, g * CH:(g + 1) * CH], u,
                            0.0, None, mybir.AluOpType.max)
                        return
                    ho = u_pool.tile([C, CH], f32, name="ho", tag="ho")
                    nc.vector.tensor_tensor(ho, po[:C, :], v,
                                            mybir.AluOpType.add)
                else:
                    if l < N_LAYERS - 1:
                        nc.scalar.activation(
                            Ht[l + 1][:, g * CH:(g + 1) * CH], po[:C, :],
                            mybir.ActivationFunctionType.Relu,
                            bias=b_sb[l], scale=1.0)
                        return
                    ho = u_pool.tile([C, CH], f32, name="ho", tag="ho")
                    nc.scalar.activation(ho, po[:C, :],
                                         mybir.ActivationFunctionType.Identity,
                                         bias=b_sb[l], scale=1.0)
                # final layer: back to natural layout and out to DRAM
                for k in range(CH // P):
                    ph = ps_tr.tile([P, P], f32, name="ph", tag="trx", bufs=2)
                    nc.tensor.matmul(ph[:, :C], ho[:, k * P:(k + 1) * P],
                                     ident[:C, :C], start=True, stop=True)
                    hb = hb_pool.tile([P, C], f32, name="hb", tag="hb")
                    nc.vector.tensor_copy(hb, ph[:, :C])
                    r0 = g * CH + k * P
                    nc.sync.dma_start(H_out[r0:r0 + P, :], hb)

            def cast_copy(eng_idx, dst, srcp):
                # 1:1 DVE:ACT — neither engine should pace the slab pipeline
                if eng_idx % 2 == 0:
                    nc.vector.tensor_copy(dst, srcp)
                else:
                    nc.scalar.copy(dst, srcp)

            def emit_half_transposes(s, h):
                """Transpose one A half-slab (8 tiles, quads into PSUM) into
                ATr columns i in [s*128, (s+1)*128), j-blocks 8h..8h+7."""
                a_sl, dma = halves[(s, h)]
                gate = nc.tensor.nop(nofuse=True)
                add_dep(gate.ins, dma.ins, True, "slab gate")
                for jq in range(2):
                    # 4 transposes into one PSUM bank -> one big copy out
                    # (copy overhead ~200ns amortizes over 64K elements)
                    pt = ps_tr.tile([P, 4 * P], f32, name="pt_a", tag="tr",
                                    bufs=2)
                    for k in range(4):
                        jb = 8 * h + 4 * jq + k
                        t = nc.tensor.matmul(
                            pt[:, k * P:(k + 1) * P],
                            a_sl[:, (4 * jq + k) * P:(4 * jq + k + 1) * P],
                            ident_a, start=True, stop=True,
                            skip_group_check=True)
                        add_dep(t.ins, gate.ins, False, "after slab gate")
                    jb0 = 8 * h + 4 * jq
                    cast_copy(2 * s + jq + h, ATr[:, jb0:jb0 + 4,
                                                  s * P:(s + 1) * P], pt)

            # agg chunk split into emission quarters for interleaving
            open_po = {}

            def emit_agg_part(l, g, mn, part):
                """part 0..3: col-packed pairs; part 3 adds self term+evac."""
                cs = slice(g * CH, (g + 1) * CH)
                if use_bf16:
                    # col-packed: even j-blocks -> partitions 0:64, odd ->
                    # 64:128; concurrent in the array's column groups
                    if part == 0:
                        po = ps_o.tile([P, CH], f32, name="po", tag="o")
                        open_po[(l, g)] = po
                    else:
                        po = open_po[(l, g)]
                    for jb in range(4 * part, 4 * part + 4):
                        h = jb % 2
                        nc.tensor.matmul(
                            po[h * C:(h + 1) * C, :], mn[:, jb, :],
                            ATr[:, jb, cs],
                            start=(jb < 2), stop=(h == 1 and jb == NB - 1),
                            skip_group_check=True)
                    if part == 3:
                        del open_po[(l, g)]
                        nc.tensor.matmul(
                            po[:C, :], wT[(l, "s")], Ht[l][:, cs],
                            start=False, stop=True, skip_group_check=True)
                        emit_evac(l, g, po)
                else:
                    if part == 0:
                        po = ps_o.tile([C, CH], f32, name="po", tag="o")
                        open_po[(l, g)] = po
                    else:
                        po = open_po[(l, g)]
                    for jb in range(4 * part, 4 * part + 4):
                        nc.tensor.matmul(
                            po, mn[:, jb, :], ATr[:, jb, cs],
                            start=(jb == 0), stop=False,
                            skip_group_check=True)
                    if part == 3:
                        del open_po[(l, g)]
                        nc.tensor.matmul(
                            po, wT[(l, "s")], Ht[l][:, cs],
                            start=False, stop=True, skip_group_check=True)
                        emit_evac(l, g, po)

            # ---- layer 1, pipelined with the A load/transpose -------------
            mns = {}
            for l in range(N_LAYERS):
                mns[l] = mn_pool.tile([P, NB, C], rdt, name="mn", tag="mn")

            # mprod L0 spread over the first slabs to keep HAM warm between
            # transpose batches (transpose-mode alone doesn't count as
            # PE-busy for the clock gate)
            for s in range(NB):
                emit_half_transposes(s, 0)
                emit_half_transposes(s, 1)
                if s < 4:
                    emit_mprod(0, mns[0], range(4 * s, 4 * s + 4))
                else:
                    # one chunk behind the transposes
                    emit_agg_part(0, (s - 4) // SPC, mns[0], (s - 4) % SPC)
            # tail chunk of layer 1 + chunk-wise mprod for layer 2
            for part in range(4):
                emit_agg_part(0, IC - 1, mns[0], part)
            emit_mprod(1, mns[1])

            # ---- layers 2..3 ---------------------------------------------
            for l in range(1, N_LAYERS):
                for g in range(IC):
                    for part in range(4):
                        emit_agg_part(l, g, mns[l], part)
                    if l + 1 < N_LAYERS:
                        jb0 = g * (NB // IC)
                        emit_mprod(l + 1, mns[l + 1],
                                   range(jb0, jb0 + NB // IC))

    return nc


# ---------------------------------------------------------------------------
# Harness entry point
# ---------------------------------------------------------------------------
_NC_CACHE = {}


def _get_nc(nn_nodes):
    if nn_nodes not in _NC_CACHE:
        _NC_CACHE[nn_nodes] = build_gcn(nn_nodes)
    return _NC_CACHE[nn_nodes]


def kernel(X, A, Wm0, Ws0, b0, Wm1, Ws1, b1, Wm2, Ws2, b2, _trace=False):
    from concourse.bass_utils import run_bass_kernel_spmd

    X = np.ascontiguousarray(np.asarray(X, dtype=np.float32))
    A = np.ascontiguousarray(np.asarray(A, dtype=np.float32))
    B, NN, _C = X.shape
    assert B == 8, f"expected batch 8 (one per core), got {B}"

    shared = {
        "Wm0": np.ascontiguousarray(np.asarray(Wm0, np.float32)),
        "Ws0": np.ascontiguousarray(np.asarray(Ws0, np.float32)),
        "b0": np.ascontiguousarray(np.asarray(b0, np.float32)),
        "Wm1": np.ascontiguousarray(np.asarray(Wm1, np.float32)),
        "Ws1": np.ascontiguousarray(np.asarray(Ws1, np.float32)),
        "b1": np.ascontiguousarray(np.asarray(b1, np.float32)),
        "Wm2": np.ascontiguousarray(np.asarray(Wm2, np.float32)),
        "Ws2": np.ascontiguousarray(np.asarray(Ws2, np.float32)),
        "b2": np.ascontiguousarray(np.asarray(b2, np.float32)),
    }
    nc = _get_nc(NN)
    in_maps = [dict(shared, X=X[b], A=A[b]) for b in range(B)]
    res = run_bass_kernel_spmd(nc, in_maps, core_ids=list(range(B)),
                               trace=_trace)
    out = np.stack([res.results[b]["H"] for b in range(B)], axis=0)
    if _trace:
        return out, res
    return out
